# Initial kernel scaffold
#
"""Trainium2 Bass kernel for DensityGCNProcessor.

Model: 2-layer GCN over a per-sample kNN graph built from 1-D density values
(K=4 nearest by |density_i - density_j|), symmetric deg^-1/2 normalization on
target indegree, relu after each layer.

Strategy
--------
kNN in a 1-D metric means: after sorting nodes by density, every node's 4
nearest neighbours lie within +/-4 sorted positions. So the whole aggregation
matrix becomes a 9-diagonal *banded* matrix in sorted order. The device kernel:

  1. transposes X^T [Cin, N] tiles on the TensorEngine and indirect-DMA
     scatters node rows into a DRAM scratch in *sorted* order (per-core rank
     window of 2048 nodes + halo),
  2. computes A1 = Band @ X_s with small banded matmuls (TensorEngine,
     float32r = full-precision fp32 at 1 cycle/row),
  3. H^T = relu(W1^T A1^T + b1) dense matmuls (channel-major),
  4. T2^T = W2^T H^T, transposed back to node-major,
  5. out = relu(Band @ T2 + b2), indirect-DMA scattered to original node order.

Host does only O(N log N) index math on the 16 KB density array: argsort, band
weights w9[r, o] (including exact reference tie-breaking by (dist, orig index),
which also reproduces the reference's duplicate-density self-target quirk), and
expands them into the per-tile band matrices.

Sharding: 8 cores = 4 batches x 2 rank-halves. Core c handles batch c//2,
sorted ranks [ (c%2)*2048, (c%2)*2048+2048 ).
"""

import numpy as np

# ---------------------------------------------------------------- constants
B = 4
CIN = 256
CHID = 512
COUT = 256
H = W = 64
N = H * W            # 4096 nodes per batch
KNN = 4
BAND = 4             # kNN lies within +/-4 sorted positions
HALF = N // 2        # 2048 ranks per core
NT1 = 17             # A1/H/T2 tiles (rows r0-4 .. r0+2172)
NT2 = 16             # output tiles  (rows r0   .. r0+2048)
XS_ROWS = HALF + 136  # 2184 scratch rows, local row l <-> rank r0 - 8 + l
SENTINEL = 1 << 20

_COMPILED = {}


# ---------------------------------------------------------------- host graph
def _build_band_weights(d_flat):
    """order [N], w9 [N, 9] f32: out_s[r] = sum_o w9[r, o+4] * g_s[r+o]."""
    order = np.argsort(d_flat, kind="stable")
    d_s = d_flat[order]

    offs = np.arange(-BAND, BAND + 1)
    ridx = np.arange(N)[:, None] + offs[None, :]
    valid = (ridx >= 0) & (ridx < N)
    ridx_c = np.clip(ridx, 0, N - 1)
    c = np.abs(d_s[ridx_c] - d_s[:, None]).astype(np.float32)
    c = np.where(valid, c, np.float32(np.inf))
    cand_j = np.where(valid, order[ridx_c], N)

    # reference = stable argsort over the full row: ties by smaller orig index.
    sel = np.lexsort((cand_j, c), axis=1)
    tgt_s = np.take_along_axis(ridx_c, sel[:, 1:KNN + 1], axis=1).reshape(-1)
    src_s = np.repeat(np.arange(N), KNN)

    deg = np.ones(N, dtype=np.float32)
    np.add.at(deg, tgt_s, np.float32(1.0))
    dinv = (np.float32(1.0) / np.sqrt(deg)).astype(np.float32)

    m = np.zeros((N, 9), dtype=np.float32)
    np.add.at(m, (tgt_s, src_s - tgt_s + BAND), np.float32(1.0))
    m[:, BAND] += 1.0  # self loops

    ro = np.arange(N)[:, None] + offs[None, :]
    rov = (ro >= 0) & (ro < N)
    w9 = m * dinv[:, None] * dinv[np.clip(ro, 0, N - 1)] * rov
    return order.astype(np.int32), w9.astype(np.float32)


def _host_graph(density_maps):
    """Per-core index/band tensors. Returns list of 8 dicts."""
    per_core = []
    for b in range(B):
        d = np.asarray(density_maps[b]).reshape(N).astype(np.float32)
        order, w9g = _build_band_weights(d)
        rank = np.empty(N, dtype=np.int64)
        rank[order] = np.arange(N)
        for half in range(2):
            r0 = half * HALF

            # scatter index: orig node j (= col of xT) -> local scratch row
            loc = rank - (r0 - 8)
            scat = np.where((loc >= 0) & (loc < XS_ROWS), loc, SENTINEL)
            scat_idx = scat.reshape(N // 128, 128).T.astype(np.int32).copy()  # [128, 32]

            # w9 rows for this core's window, zero outside usable range
            # w9_dev[i] = w9 at rank (r0 - 4 + i), i in [0, NT1*128)
            w9_dev = np.zeros((NT1 * 128, 9), dtype=np.float32)
            g = np.arange(NT1 * 128) + (r0 - 4)
            ok = (g >= 0) & (g < N) & (g < r0 + HALF + 4)
            w9_dev[ok] = w9g[g[ok]]

            # band matrices bandT[k, q, r]: k<17 -> L1 tile (out rows r0-4+128k+r),
            # k>=17 -> L2 tile (out rows r0+128(k-17)+r). value = w9row[q - r].
            bandT = np.zeros((NT1 + NT2, 136, 128), dtype=np.float32)
            qq = np.arange(136)[:, None]
            rr = np.arange(128)[None, :]
            dd = qq - rr
            okd = (dd >= 0) & (dd < 9)
            dd_c = np.clip(dd, 0, 8)
            for k in range(NT1 + NT2):
                if k < NT1:
                    rows = w9_dev[128 * k + rr[0]]          # [128, 9]
                else:
                    rows = w9_dev[4 + 128 * (k - NT1) + rr[0]]
                bandT[k] = np.where(okd, rows[rr[0][None, :].repeat(136, 0) * 0 + rr, dd_c][0] if False else rows[rr, dd_c], 0.0)[0] if False else np.where(okd, rows[np.broadcast_to(rr, (136, 128)), dd_c], 0.0)

            # output scatter: (p, t') -> orig index of rank r0 + 128 t' + p
            out_idx = order[r0 + (np.arange(NT2)[None, :] * 128 + np.arange(128)[:, None])].astype(np.int32).copy()

            per_core.append(dict(scat_idx=scat_idx, bandT=bandT, out_idx=out_idx,
                                 order=order, rank=rank))
    return per_core


# ---------------------------------------------------------------- device IR
def build_nc():
    import concourse.bass as bass
    import concourse.mybir as mybir
    from concourse.tile import TileContext
    from concourse.masks import make_identity

    F32 = mybir.dt.float32
    F32R = mybir.dt.float32r
    I32 = mybir.dt.int32

    def r(ap):
        return ap.bitcast(F32R)

    nc = bass.Bass()
    xT = nc.dram_tensor("xT", [CIN, N], F32, kind="ExternalInput")
    w1 = nc.dram_tensor("w1", [CIN, CHID], F32, kind="ExternalInput")
    w2 = nc.dram_tensor("w2", [CHID, COUT], F32, kind="ExternalInput")
    b1 = nc.dram_tensor("b1", [CHID], F32, kind="ExternalInput")
    b2rep = nc.dram_tensor("b2rep", [128, COUT], F32, kind="ExternalInput")
    bandT = nc.dram_tensor("bandT", [NT1 + NT2, 136, 128], F32, kind="ExternalInput")
    scat_idx = nc.dram_tensor("scat_idx", [128, N // 128], I32, kind="ExternalInput")
    out_idx = nc.dram_tensor("out_idx", [128, NT2], I32, kind="ExternalInput")
    out_nodes = nc.dram_tensor("out_nodes", [N, COUT], F32, kind="ExternalOutput")
    xs = nc.dram_tensor("xs", [XS_ROWS, CIN], F32, kind="Internal")

    NJ = N // 128  # 32 node-column tiles of xT

    with TileContext(nc) as tc:
        with (
            tc.tile_pool(name="const", bufs=1) as cpool,
            tc.tile_pool(name="big", bufs=1) as big,
            tc.tile_pool(name="stream", bufs=3) as sp,
            tc.tile_pool(name="psum", bufs=2, space="PSUM") as pp,
        ):
            ident = cpool.tile([128, 128], F32)
            make_identity(nc, ident)
            zero_sb = cpool.tile([128, CIN], F32)
            nc.gpsimd.memset(zero_sb, 0.0)

            w1_sb = cpool.tile([128, 2, CHID], F32)   # [k-part, k-chunk, m]
            nc.sync.dma_start(w1_sb, w1.rearrange("(c p) m -> p c m", p=128))
            w2_sb = cpool.tile([128, 4, COUT], F32)
            nc.sync.dma_start(w2_sb, w2.rearrange("(c p) m -> p c m", p=128))
            b1_sb = cpool.tile([128, 4], F32)
            nc.sync.dma_start(b1_sb, b1.rearrange("(c p) -> p c", p=128))
            b2_sb = cpool.tile([128, COUT], F32)
            nc.sync.dma_start(b2_sb, b2rep[:, :])
            scat_sb = cpool.tile([128, NJ], I32)
            nc.sync.dma_start(scat_sb, scat_idx[:, :])
            oidx_sb = cpool.tile([128, NT2], I32)
            nc.sync.dma_start(oidx_sb, out_idx[:, :])

            # ---------------- phase X: transpose X^T -> node-major, scatter sorted
            xnode = big.tile([128, NJ, CIN], F32)  # 4 MB
            for jt in range(NJ):
                for cb in range(2):
                    xt_sb = sp.tile([128, 128], F32, tag="xt")
                    nc.sync.dma_start(xt_sb, xT[128 * cb:128 * (cb + 1),
                                                128 * jt:128 * (jt + 1)])
                    tp = pp.tile([128, 128], F32, tag="tp", space="PSUM")
                    nc.tensor.transpose(tp, xt_sb, ident)
                    nc.vector.tensor_copy(xnode[:, jt, 128 * cb:128 * (cb + 1)], tp)

            # zero rows that scatter may skip (rank outside [0, N))
            nc.sync.dma_start(xs[0:8, :], zero_sb[0:8, :])
            nc.sync.dma_start(xs[HALF + 8:XS_ROWS, :], zero_sb[:, :])
            nc.gpsimd.indirect_dma_start(
                out=xs[:, :],
                out_offset=bass.IndirectOffsetOnAxis(ap=scat_sb[:, :], axis=0),
                in_=xnode[:, :, :],
                in_offset=None,
                bounds_check=XS_ROWS - 1,
                oob_is_err=False,
            )

            # ---------------- L1 aggregation: A1 = Band1 @ X_s   (node-major)
            a1T = big.tile([128, 2, NT1 * 128], F32)   # A1^T, cin-chunk major
            for t in range(NT1):
                rhs0 = sp.tile([128, CIN], F32, tag="rhs0")
                nc.sync.dma_start(rhs0, xs[128 * t:128 * t + 128, :])
                rhs1 = sp.tile([8, CIN], F32, tag="rhs1")
                nc.sync.dma_start(rhs1, xs[128 * t + 128:128 * t + 136, :])
                bA = sp.tile([128, 128], F32, tag="bA")
                nc.sync.dma_start(bA, bandT[t, 0:128, :])
                bB = sp.tile([8, 128], F32, tag="bB")
                nc.sync.dma_start(bB, bandT[t, 128:136, :])
                psA = pp.tile([128, CIN], F32, tag="psA", space="PSUM")
                nc.tensor.matmul(psA, lhsT=r(bA), rhs=r(rhs0), start=True, stop=False)
                nc.tensor.matmul(psA, lhsT=r(bB), rhs=r(rhs1), start=False, stop=True)
                a1_sb = sp.tile([128, CIN], F32, tag="a1")
                nc.vector.tensor_copy(a1_sb, psA)
                for cb in range(2):
                    tpa = pp.tile([128, 128], F32, tag="tp", space="PSUM")
                    nc.tensor.transpose(tpa, a1_sb[:, 128 * cb:128 * (cb + 1)], ident)
                    nc.vector.tensor_copy(a1T[:, cb, 128 * t:128 * t + 128], tpa)

            # ---------------- L1 dense: H^T = relu(W1^T A1^T + b1)
            NODES = NT1 * 128
            blocks = [(i, min(i + 512, NODES)) for i in range(0, NODES, 512)]
            hT = big.tile([128, 4, NODES], F32)
            for lo, hi in blocks:
                for mb in range(4):
                    psH = pp.tile([128, 512], F32, tag="psH", space="PSUM")
                    for kb in range(2):
                        nc.tensor.matmul(
                            psH[:, 0:hi - lo],
                            lhsT=r(w1_sb[:, kb, 128 * mb:128 * (mb + 1)]),
                            rhs=r(a1T[:, kb, lo:hi]),
                            start=(kb == 0), stop=(kb == 1))
                    nc.scalar.activation(
                        hT[:, mb, lo:hi], psH[:, 0:hi - lo],
                        mybir.ActivationFunctionType.Relu,
                        bias=b1_sb[:, mb:mb + 1], scale=1.0)

            # ---------------- L2 dense: T2^T = W2^T H^T
            t2T = big.tile([128, 2, NODES], F32)
            for lo, hi in blocks:
                for mb in range(2):
                    psT = pp.tile([128, 512], F32, tag="psH", space="PSUM")
                    for kb in range(4):
                        nc.tensor.matmul(
                            psT[:, 0:hi - lo],
                            lhsT=r(w2_sb[:, kb, 128 * mb:128 * (mb + 1)]),
                            rhs=r(hT[:, kb, lo:hi]),
                            start=(kb == 0), stop=(kb == 3))
                    nc.vector.tensor_copy(t2T[:, mb, lo:hi], psT[:, 0:hi - lo])

            # ---------------- transpose T2 back to node-major
            t2n = big.tile([128, NT1, COUT], F32)
            for t in range(NT1):
                for cb in range(2):
                    tpb = pp.tile([128, 128], F32, tag="tp", space="PSUM")
                    nc.tensor.transpose(tpb, t2T[:, cb, 128 * t:128 * t + 128], ident)
                    nc.vector.tensor_copy(t2n[:, t, 128 * cb:128 * (cb + 1)], tpb)

            # ---------------- L2 aggregation + bias + relu, then scatter out
            out_all = big.tile([128, NT2, COUT], F32)
            for t in range(NT2):
                bA2 = sp.tile([128, 128], F32, tag="bA")
                nc.sync.dma_start(bA2, bandT[NT1 + t, 0:128, :])
                bB2 = sp.tile([8, 128], F32, tag="bB")
                nc.sync.dma_start(bB2, bandT[NT1 + t, 128:136, :])
                psO = pp.tile([128, COUT], F32, tag="psA", space="PSUM")
                nc.tensor.matmul(psO, lhsT=r(bA2), rhs=r(t2n[:, t, :]),
                                 start=True, stop=False)
                nc.tensor.matmul(psO, lhsT=r(bB2), rhs=r(t2n[0:8, t + 1, :]),
                                 start=False, stop=True)
                nc.vector.tensor_tensor(out=out_all[:, t, :], in0=psO, in1=b2_sb,
                                        op=mybir.AluOpType.add)
                nc.vector.tensor_scalar(out=out_all[:, t, :], in0=out_all[:, t, :],
                                        scalar1=0.0, scalar2=None,
                                        op0=mybir.AluOpType.max)

            nc.gpsimd.indirect_dma_start(
                out=out_nodes[:, :],
                out_offset=bass.IndirectOffsetOnAxis(ap=oidx_sb[:, :], axis=0),
                in_=out_all[:, :, :],
                in_offset=None,
                bounds_check=N - 1,
                oob_is_err=False,
            )

    return nc


def make_in_maps(density_maps, feature_maps, W1, b1, W2, b2):
    graph = _host_graph(density_maps)
    fm = np.ascontiguousarray(np.asarray(feature_maps, dtype=np.float32))
    W1 = np.ascontiguousarray(np.asarray(W1, dtype=np.float32))
    W2 = np.ascontiguousarray(np.asarray(W2, dtype=np.float32))
    b1 = np.ascontiguousarray(np.asarray(b1, dtype=np.float32))
    b2r = np.broadcast_to(np.asarray(b2, dtype=np.float32), (128, COUT)).copy()
    in_maps = []
    for c in range(8):
        g = graph[c]
        in_maps.append({
            "xT": fm[c // 2].reshape(CIN, N),
            "w1": W1, "w2": W2, "b1": b1, "b2rep": b2r,
            "bandT": g["bandT"], "scat_idx": g["scat_idx"], "out_idx": g["out_idx"],
        })
    return in_maps, graph


def kernel(density_maps, feature_maps, W1, b1, W2, b2):
    from concourse.bass_utils import run_bass_kernel_spmd

    if "nc" not in _COMPILED:
        _COMPILED["nc"] = build_nc()
    nc = _COMPILED["nc"]

    in_maps, graph = make_in_maps(density_maps, feature_maps, W1, b1, W2, b2)
    res = run_bass_kernel_spmd(nc, in_maps, core_ids=list(range(8)))

    out = np.empty((B, N, COUT), dtype=np.float32)
    for b in range(B):
        o0 = res.results[2 * b]["out_nodes"]
        o1 = res.results[2 * b + 1]["out_nodes"]
        mask = (graph[2 * b]["rank"] < HALF)[:, None]
        out[b] = np.where(mask, o0, o1)
    return np.ascontiguousarray(
        out.reshape(B, H, W, COUT).transpose(0, 3, 1, 2)).astype(np.float32)


# revision 2
# speedup vs baseline: 1.9491x; 1.9491x over previous
"""Trainium2 Bass kernel for DensityGCNProcessor.

Model: 2-layer GCN over a per-sample kNN graph built from 1-D density values
(K=4 nearest by |density_i - density_j|), symmetric deg^-1/2 normalization on
target indegree, relu after each layer.

Strategy
--------
kNN in a 1-D metric means: after sorting nodes by density, every node's 4
nearest neighbours lie within +/-4 sorted positions. So the whole aggregation
matrix becomes a 9-diagonal *banded* matrix in sorted order. The device kernel:

  1. transposes X^T [Cin, N] tiles on the TensorEngine and indirect-DMA
     scatters node rows into a DRAM scratch in *sorted* order (per-core rank
     window of 2048 nodes + halo),
  2. computes A1 = Band @ X_s with small banded matmuls (TensorEngine,
     float32r = full-precision fp32 at 1 cycle/row),
  3. H^T = relu(W1^T A1^T + b1) dense matmuls (channel-major),
  4. T2^T = W2^T H^T, transposed back to node-major,
  5. out = relu(Band @ T2 + b2), indirect-DMA scattered to original node order.

Host does only O(N log N) index math on the 16 KB density array: argsort, band
weights w9[r, o] (including exact reference tie-breaking by (dist, orig index),
which also reproduces the reference's duplicate-density self-target quirk), and
expands them into the per-tile band matrices.

Sharding: 8 cores = 4 batches x 2 rank-halves. Core c handles batch c//2,
sorted ranks [ (c%2)*2048, (c%2)*2048+2048 ).
"""

import numpy as np

# ---------------------------------------------------------------- constants
B = 4
CIN = 256
CHID = 512
COUT = 256
H = W = 64
N = H * W            # 4096 nodes per batch
KNN = 4
BAND = 4             # kNN lies within +/-4 sorted positions
HALF = N // 2        # 2048 ranks per core
NT1 = 17             # A1/H/T2 tiles (rows r0-4 .. r0+2172)
NT2 = 16             # output tiles  (rows r0   .. r0+2048)
XS_ROWS = HALF + 136  # 2184 scratch rows, local row l <-> rank r0 - 8 + l
SENTINEL = 1 << 20

_COMPILED = {}


# ---------------------------------------------------------------- host graph
def _build_band_weights(d_flat):
    """order [N], w9 [N, 9] f32: out_s[r] = sum_o w9[r, o+4] * g_s[r+o]."""
    order = np.argsort(d_flat, kind="stable")
    d_s = d_flat[order]

    offs = np.arange(-BAND, BAND + 1)
    ridx = np.arange(N)[:, None] + offs[None, :]
    valid = (ridx >= 0) & (ridx < N)
    ridx_c = np.clip(ridx, 0, N - 1)
    c = np.abs(d_s[ridx_c] - d_s[:, None]).astype(np.float32)
    c = np.where(valid, c, np.float32(np.inf))
    cand_j = np.where(valid, order[ridx_c], N)

    # reference = stable argsort over the full row: ties by smaller orig index.
    sel = np.lexsort((cand_j, c), axis=1)
    tgt_s = np.take_along_axis(ridx_c, sel[:, 1:KNN + 1], axis=1).reshape(-1)
    src_s = np.repeat(np.arange(N), KNN)

    deg = np.ones(N, dtype=np.float32)
    np.add.at(deg, tgt_s, np.float32(1.0))
    dinv = (np.float32(1.0) / np.sqrt(deg)).astype(np.float32)

    m = np.zeros((N, 9), dtype=np.float32)
    np.add.at(m, (tgt_s, src_s - tgt_s + BAND), np.float32(1.0))
    m[:, BAND] += 1.0  # self loops

    ro = np.arange(N)[:, None] + offs[None, :]
    rov = (ro >= 0) & (ro < N)
    w9 = m * dinv[:, None] * dinv[np.clip(ro, 0, N - 1)] * rov
    return order.astype(np.int32), w9.astype(np.float32)


def _host_graph(density_maps):
    """Per-core index/band tensors. Returns list of 8 dicts."""
    per_core = []
    for b in range(B):
        d = np.asarray(density_maps[b]).reshape(N).astype(np.float32)
        order, w9g = _build_band_weights(d)
        rank = np.empty(N, dtype=np.int64)
        rank[order] = np.arange(N)
        for half in range(2):
            r0 = half * HALF

            # scatter index: orig node j (= col of xT) -> local scratch row
            loc = rank - (r0 - 8)
            scat = np.where((loc >= 0) & (loc < XS_ROWS), loc, SENTINEL)
            scat_idx = scat.reshape(N // 128, 128).T.astype(np.int32).copy()  # [128, 32]

            # w9 rows for this core's window, zero outside usable range
            # w9_dev[i] = w9 at rank (r0 - 4 + i), i in [0, NT1*128)
            w9_dev = np.zeros((NT1 * 128, 9), dtype=np.float32)
            g = np.arange(NT1 * 128) + (r0 - 4)
            ok = (g >= 0) & (g < N) & (g < r0 + HALF + 4)
            w9_dev[ok] = w9g[g[ok]]

            # band matrices bandT[k, q, r]: k<17 -> L1 tile (out rows r0-4+128k+r),
            # k>=17 -> L2 tile (out rows r0+128(k-17)+r). value = w9row[q - r].
            bandT = np.zeros((NT1 + NT2, 136, 128), dtype=np.float32)
            qq = np.arange(136)[:, None]          # window position
            rr = np.arange(128)[None, :]          # out row within tile
            dd = qq - rr                          # w9 column (o + 4)
            okd = (dd >= 0) & (dd < 9)
            dd_c = np.clip(dd, 0, 8)
            rr_b = np.broadcast_to(rr, (136, 128))
            for k in range(NT1 + NT2):
                base = 128 * k if k < NT1 else 4 + 128 * (k - NT1)
                rows = w9_dev[base + np.arange(128)]          # [128, 9]
                bandT[k] = np.where(okd, rows[rr_b, dd_c], 0.0)

            # output scatter: (p, t') -> orig index of rank r0 + 128 t' + p
            out_idx = order[r0 + (np.arange(NT2)[None, :] * 128 + np.arange(128)[:, None])].astype(np.int32).copy()

            per_core.append(dict(scat_idx=scat_idx, bandT=bandT, out_idx=out_idx,
                                 order=order, rank=rank))
    return per_core


# ---------------------------------------------------------------- device IR
def build_nc():
    import concourse.bass as bass
    import concourse.mybir as mybir
    from concourse.tile import TileContext
    from concourse.masks import make_identity

    F32 = mybir.dt.float32
    F32R = mybir.dt.float32r
    I32 = mybir.dt.int32

    def r(ap):
        return ap.bitcast(F32R)

    nc = bass.Bass()
    xT = nc.dram_tensor("xT", [CIN, N], F32, kind="ExternalInput")
    w1 = nc.dram_tensor("w1", [CIN, CHID], F32, kind="ExternalInput")
    w2 = nc.dram_tensor("w2", [CHID, COUT], F32, kind="ExternalInput")
    b1 = nc.dram_tensor("b1", [CHID], F32, kind="ExternalInput")
    b2rep = nc.dram_tensor("b2rep", [128, COUT], F32, kind="ExternalInput")
    bandT = nc.dram_tensor("bandT", [NT1 + NT2, 136, 128], F32, kind="ExternalInput")
    scat_idx = nc.dram_tensor("scat_idx", [128, N // 128], I32, kind="ExternalInput")
    out_idx = nc.dram_tensor("out_idx", [128, NT2], I32, kind="ExternalInput")
    out_nodes = nc.dram_tensor("out_nodes", [N, COUT], F32, kind="ExternalOutput")
    xs = nc.dram_tensor("xs", [XS_ROWS, CIN], F32, kind="Internal")

    NJ = N // 128  # 32 node-column tiles of xT

    with TileContext(nc) as tc:
        with (
            tc.tile_pool(name="const", bufs=1) as cpool,
            tc.tile_pool(name="big", bufs=1) as big,
            tc.tile_pool(name="stream", bufs=3) as sp,
            tc.tile_pool(name="psum", bufs=2, space="PSUM") as pp,
        ):
            ident = cpool.tile([128, 128], F32)
            make_identity(nc, ident)
            zero_sb = cpool.tile([128, CIN], F32)
            nc.gpsimd.memset(zero_sb, 0.0)

            w1_sb = cpool.tile([128, 2, CHID], F32)   # [k-part, k-chunk, m]
            nc.sync.dma_start(w1_sb, w1.rearrange("(c p) m -> p c m", p=128))
            w2_sb = cpool.tile([128, 4, COUT], F32)
            nc.sync.dma_start(w2_sb, w2.rearrange("(c p) m -> p c m", p=128))
            b1_sb = cpool.tile([128, 4], F32)
            nc.sync.dma_start(b1_sb, b1.rearrange("(c p) -> p c", p=128))
            b2_sb = cpool.tile([128, COUT], F32)
            nc.sync.dma_start(b2_sb, b2rep[:, :])
            scat_sb = cpool.tile([128, NJ], I32)
            nc.sync.dma_start(scat_sb, scat_idx[:, :])
            oidx_sb = cpool.tile([128, NT2], I32)
            nc.sync.dma_start(oidx_sb, out_idx[:, :])

            # ---------------- phase X: transpose X^T -> node-major, scatter sorted
            xnode = big.tile([128, NJ, CIN], F32)  # 4 MB
            for jt in range(NJ):
                for cb in range(2):
                    xt_sb = sp.tile([128, 128], F32, tag="xt")
                    nc.sync.dma_start(xt_sb, xT[128 * cb:128 * (cb + 1),
                                                128 * jt:128 * (jt + 1)])
                    tp = pp.tile([128, 128], F32, tag="tp", space="PSUM")
                    nc.tensor.transpose(tp, xt_sb, ident)
                    nc.vector.tensor_copy(xnode[:, jt, 128 * cb:128 * (cb + 1)], tp)

            # zero rows that scatter may skip (rank outside [0, N))
            nc.sync.dma_start(xs[0:8, :], zero_sb[0:8, :])
            nc.sync.dma_start(xs[HALF + 8:XS_ROWS, :], zero_sb[:, :])
            nc.gpsimd.indirect_dma_start(
                out=xs[:, :],
                out_offset=bass.IndirectOffsetOnAxis(ap=scat_sb[:, :], axis=0),
                in_=xnode[:, :, :],
                in_offset=None,
                bounds_check=XS_ROWS - 1,
                oob_is_err=False,
            )

            # ---------------- L1 aggregation: A1 = Band1 @ X_s   (node-major)
            a1T = big.tile([128, 2, NT1 * 128], F32)   # A1^T, cin-chunk major
            for t in range(NT1):
                rhs0 = sp.tile([128, CIN], F32, tag="rhs0")
                nc.sync.dma_start(rhs0, xs[128 * t:128 * t + 128, :])
                rhs1 = sp.tile([8, CIN], F32, tag="rhs1")
                nc.sync.dma_start(rhs1, xs[128 * t + 128:128 * t + 136, :])
                bA = sp.tile([128, 128], F32, tag="bA")
                nc.sync.dma_start(bA, bandT[t, 0:128, :])
                bB = sp.tile([8, 128], F32, tag="bB")
                nc.sync.dma_start(bB, bandT[t, 128:136, :])
                psA = pp.tile([128, CIN], F32, tag="psA", space="PSUM")
                nc.tensor.matmul(psA, lhsT=r(bA), rhs=r(rhs0), start=True, stop=False)
                nc.tensor.matmul(psA, lhsT=r(bB), rhs=r(rhs1), start=False, stop=True)
                a1_sb = sp.tile([128, CIN], F32, tag="a1")
                nc.vector.tensor_copy(a1_sb, psA)
                for cb in range(2):
                    tpa = pp.tile([128, 128], F32, tag="tp", space="PSUM")
                    nc.tensor.transpose(tpa, a1_sb[:, 128 * cb:128 * (cb + 1)], ident)
                    nc.vector.tensor_copy(a1T[:, cb, 128 * t:128 * t + 128], tpa)

            # ---------------- L1 dense: H^T = relu(W1^T A1^T + b1)
            NODES = NT1 * 128
            blocks = [(i, min(i + 512, NODES)) for i in range(0, NODES, 512)]
            hT = big.tile([128, 4, NODES], F32)
            for lo, hi in blocks:
                for mb in range(4):
                    psH = pp.tile([128, 512], F32, tag="psH", space="PSUM")
                    for kb in range(2):
                        nc.tensor.matmul(
                            psH[:, 0:hi - lo],
                            lhsT=r(w1_sb[:, kb, 128 * mb:128 * (mb + 1)]),
                            rhs=r(a1T[:, kb, lo:hi]),
                            start=(kb == 0), stop=(kb == 1))
                    nc.scalar.activation(
                        hT[:, mb, lo:hi], psH[:, 0:hi - lo],
                        mybir.ActivationFunctionType.Relu,
                        bias=b1_sb[:, mb:mb + 1], scale=1.0)

            # ---------------- L2 dense: T2^T = W2^T H^T
            t2T = big.tile([128, 2, NODES], F32)
            for lo, hi in blocks:
                for mb in range(2):
                    psT = pp.tile([128, 512], F32, tag="psH", space="PSUM")
                    for kb in range(4):
                        nc.tensor.matmul(
                            psT[:, 0:hi - lo],
                            lhsT=r(w2_sb[:, kb, 128 * mb:128 * (mb + 1)]),
                            rhs=r(hT[:, kb, lo:hi]),
                            start=(kb == 0), stop=(kb == 3))
                    nc.vector.tensor_copy(t2T[:, mb, lo:hi], psT[:, 0:hi - lo])

            # ---------------- transpose T2 back to node-major
            t2n = big.tile([128, NT1, COUT], F32)
            for t in range(NT1):
                for cb in range(2):
                    tpb = pp.tile([128, 128], F32, tag="tp", space="PSUM")
                    nc.tensor.transpose(tpb, t2T[:, cb, 128 * t:128 * t + 128], ident)
                    nc.vector.tensor_copy(t2n[:, t, 128 * cb:128 * (cb + 1)], tpb)

            # ---------------- L2 aggregation + bias + relu, then scatter out
            out_all = big.tile([128, NT2, COUT], F32)
            for t in range(NT2):
                bA2 = sp.tile([128, 128], F32, tag="bA")
                nc.sync.dma_start(bA2, bandT[NT1 + t, 0:128, :])
                bB2 = sp.tile([8, 128], F32, tag="bB")
                nc.sync.dma_start(bB2, bandT[NT1 + t, 128:136, :])
                psO = pp.tile([128, COUT], F32, tag="psA", space="PSUM")
                nc.tensor.matmul(psO, lhsT=r(bA2), rhs=r(t2n[:, t, :]),
                                 start=True, stop=False)
                nc.tensor.matmul(psO, lhsT=r(bB2), rhs=r(t2n[0:8, t + 1, :]),
                                 start=False, stop=True)
                nc.vector.tensor_tensor(out=out_all[:, t, :], in0=psO, in1=b2_sb,
                                        op=mybir.AluOpType.add)
                nc.vector.tensor_scalar(out=out_all[:, t, :], in0=out_all[:, t, :],
                                        scalar1=0.0, scalar2=None,
                                        op0=mybir.AluOpType.max)

            nc.gpsimd.indirect_dma_start(
                out=out_nodes[:, :],
                out_offset=bass.IndirectOffsetOnAxis(ap=oidx_sb[:, :], axis=0),
                in_=out_all[:, :, :],
                in_offset=None,
                bounds_check=N - 1,
                oob_is_err=False,
            )

    return nc


def make_in_maps(density_maps, feature_maps, W1, b1, W2, b2):
    graph = _host_graph(density_maps)
    fm = np.ascontiguousarray(np.asarray(feature_maps, dtype=np.float32))
    W1 = np.ascontiguousarray(np.asarray(W1, dtype=np.float32))
    W2 = np.ascontiguousarray(np.asarray(W2, dtype=np.float32))
    b1 = np.ascontiguousarray(np.asarray(b1, dtype=np.float32))
    b2r = np.broadcast_to(np.asarray(b2, dtype=np.float32), (128, COUT)).copy()
    in_maps = []
    for c in range(8):
        g = graph[c]
        in_maps.append({
            "xT": fm[c // 2].reshape(CIN, N),
            "w1": W1, "w2": W2, "b1": b1, "b2rep": b2r,
            "bandT": g["bandT"], "scat_idx": g["scat_idx"], "out_idx": g["out_idx"],
        })
    return in_maps, graph


def kernel(density_maps, feature_maps, W1, b1, W2, b2):
    from concourse.bass_utils import run_bass_kernel_spmd

    if "nc" not in _COMPILED:
        _COMPILED["nc"] = build_nc()
    nc = _COMPILED["nc"]

    in_maps, graph = make_in_maps(density_maps, feature_maps, W1, b1, W2, b2)
    res = run_bass_kernel_spmd(nc, in_maps, core_ids=list(range(8)))

    out = np.empty((B, N, COUT), dtype=np.float32)
    for b in range(B):
        o0 = res.results[2 * b]["out_nodes"]
        o1 = res.results[2 * b + 1]["out_nodes"]
        mask = (graph[2 * b]["rank"] < HALF)[:, None]
        out[b] = np.where(mask, o0, o1)
    return np.ascontiguousarray(
        out.reshape(B, H, W, COUT).transpose(0, 3, 1, 2)).astype(np.float32)
